# Initial kernel scaffold
#
"""Trainium2 Bass kernel for nn_InterpolationModel (NaN-gap linear interpolation).

Problem: x [256, 2048, 22, 2] f32, one contiguous NaN gap along T per batch row.
Output: x with the gap filled by linear interpolation between the last valid
frame before the gap (s) and the first valid frame after it (e).

Strategy (pure data parallel over batch, 32 rows per core):
  - Bulk copy x -> y through SBUF in 4 chunks of 8 rows ([128, 5632] tiles,
    partition = 128 consecutive frames, 22.5KB contiguous per partition).
  - While each chunk transits SBUF, sample element 0 of every frame
    (stride-44 AP) and reduce per partition: min(t + 65536*valid) and
    min((valid-1)*t) -> first/last NaN frame per partition.
  - One TensorE transpose + grouped reduce turns per-partition partials into
    per-row s (last valid before gap), e (first valid after), 1/(e-s).
  - Per-row scalars round-trip through a tiny DRAM scratch so an indirect
    gather can replicate them to a [128, .] layout (4 partitions per row).
  - Fixed 512-frame window starting at s+1 always covers the whole gap
    (gap <= 511) and never leaves the row (s < 1024 => s+513 <= 1536 < 2048).
    Gather the window [128, 5632], compute interp = xs + (t-s)*slope with
    broadcast APs, keep original values where not NaN (copy_predicated),
    scatter back over y at the same offsets.

Everything is a single Tile-scheduled program; the only cross-phase sync
needed beyond Tile's tracking is "scatter after bulk stores" and
"scalar gather after scratch write", wired with add_dep_helper.
"""

import numpy as np

import concourse.bacc as bacc
import concourse.bass as bass
import concourse.mybir as mybir
import concourse.tile as tile
from bass_rust import add_dep_helper
from concourse.masks import make_identity

F32 = mybir.dt.float32
I32 = mybir.dt.int32
AX = mybir.AxisListType
OP = mybir.AluOpType

# Full problem: B=256, T=2048, A=22, D=2 over 8 cores.
B, T, A, D = 256, 2048, 22, 2
C = A * D            # 44 contiguous f32 per frame
NCORES = 8
R = B // NCORES      # 32 rows per core
CHUNKS = 4           # bulk-copy chunks per core
RCH = R // CHUNKS    # 8 rows per chunk
P = 128
FPP = (RCH * T) // P  # 128 frames per partition in a chunk tile
PPR = T // FPP        # 16 partitions per row
WSUB = 4              # window partitions per row (4 * 32 rows = 128)
WF = 128              # frames per window partition (window = 512 frames)
BIG = 65536.0


def _bcast_mid(ap, count):
    """[P, n] AP -> [P, count, n] with a stride-0 middle axis."""
    return bass.AP(ap.tensor, ap.offset, [list(ap.ap[0]), [0, count], list(ap.ap[1])])


def _bcast_inner(ap, count):
    """[P, n] AP -> [P, n, count] with a stride-0 inner axis."""
    return bass.AP(ap.tensor, ap.offset, [list(ap.ap[0]), list(ap.ap[1]), [0, count]])


def _ins(bi):
    return bi.ins if hasattr(bi, "ins") else bi


def build_kernel(tc, x, y):
    nc = tc.nc
    xv = x.rearrange("b t c -> (b t) c")   # [R*T, C] frame rows, offset 0
    yv = y.rearrange("b t c -> (b t) c")

    from contextlib import ExitStack

    with ExitStack() as ctx:
        const = ctx.enter_context(tc.tile_pool(name="const", bufs=1))
        data = ctx.enter_context(tc.tile_pool(name="data", bufs=3))
        det = ctx.enter_context(tc.tile_pool(name="det", bufs=2))
        small = ctx.enter_context(tc.tile_pool(name="small", bufs=1))
        win = ctx.enter_context(tc.tile_pool(name="win", bufs=1))
        psum = ctx.enter_context(tc.tile_pool(name="psum", bufs=1, space="PSUM"))
        dram = ctx.enter_context(tc.tile_pool(name="dram", bufs=1, space="DRAM"))
        # ---- constants (built on device) ----
        ident = const.tile([P, P], F32)
        make_identity(nc, ident[:])
        # I128f[p, j] = 128*p + j  (= row_in_chunk*2048 + t_in_row)
        i128f = const.tile([P, FPP], F32)
        nc.gpsimd.iota(i128f[:], pattern=[[1, FPP]], base=0,
                       channel_multiplier=FPP,
                       allow_small_or_imprecise_dtypes=True)
        # Fgrid[p, f] = f
        fgrid = const.tile([P, WF], F32)
        nc.gpsimd.iota(fgrid[:], pattern=[[1, WF]], base=0,
                       channel_multiplier=0,
                       allow_small_or_imprecise_dtypes=True)
        # C2048[c, i] = 2048 * i   (row-in-chunk offset correction)
        c2048 = const.tile([CHUNKS, RCH], F32)
        nc.gpsimd.iota(c2048[:], pattern=[[T, RCH]], base=0,
                       channel_multiplier=0,
                       allow_small_or_imprecise_dtypes=True)
        # per-partition helpers
        pidx = const.tile([P, 1], I32)
        nc.gpsimd.iota(pidx[:], pattern=[[1, 1]], base=0, channel_multiplier=1)
        idx4 = const.tile([P, 1], I32)          # p // 4  (scratch gather idx)
        nc.vector.tensor_scalar(out=idx4[:], in0=pidx[:], scalar1=2,
                                scalar2=None, op0=OP.arith_shift_right)
        d128i = const.tile([P, 1], I32)
        nc.vector.tensor_scalar(out=d128i[:], in0=idx4[:], scalar1=11,
                                scalar2=None, op0=OP.arith_shift_left)
        d128f = const.tile([P, 1], F32)         # (p//4) * 2048
        nc.vector.tensor_copy(out=d128f[:], in_=d128i[:])
        pm4 = const.tile([P, 1], I32)
        nc.vector.tensor_scalar(out=pm4[:], in0=pidx[:], scalar1=3,
                                scalar2=None, op0=OP.bitwise_and)
        w128i = const.tile([P, 1], I32)
        nc.vector.tensor_scalar(out=w128i[:], in0=pm4[:], scalar1=128, scalar2=1,
                                op0=OP.mult, op1=OP.add)
        w128f = const.tile([P, 1], F32)         # 1 + 128*(p%4)
        nc.vector.tensor_copy(out=w128f[:], in_=w128i[:])

        # ---- bulk copy + per-partition gap detection ----
        m = small.tile([P, 2 * CHUNKS], F32)    # cols 0-3 min-partials, 4-7 neg-max
        stores = []
        for k in range(CHUNKS):
            xk = data.tile([P, FPP * C], F32)
            src = x[k * RCH:(k + 1) * RCH].rearrange(
                "r (q j) c -> (r q) (j c)", q=PPR)
            nc.sync.dma_start(out=xk[:], in_=src)

            samp = xk[:].rearrange("p (j c) -> p j c", c=C)[:, :, 0:1]
            v = det.tile([P, FPP], F32)
            nc.vector.tensor_tensor(
                out=v[:].rearrange("p (j o) -> p j o", o=1),
                in0=samp, in1=samp, op=OP.is_equal)
            bb = det.tile([P, FPP], F32)
            # valid*BIG + t_chunk : min over j = first NaN t (per partition)
            nc.vector.scalar_tensor_tensor(
                out=bb[:], in0=v[:], scalar=BIG, in1=i128f[:],
                op0=OP.mult, op1=OP.add)
            nc.vector.tensor_reduce(out=m[:, k:k + 1], in_=bb[:],
                                    axis=AX.X, op=OP.min)
            dd = det.tile([P, FPP], F32)
            # (valid-1)*t_chunk = -(t on NaN frames) : min over j = -last NaN t
            nc.vector.scalar_tensor_tensor(
                out=dd[:], in0=v[:], scalar=-1.0, in1=i128f[:],
                op0=OP.add, op1=OP.mult)
            nc.vector.tensor_reduce(out=m[:, CHUNKS + k:CHUNKS + k + 1],
                                    in_=dd[:], axis=AX.X, op=OP.min)

            dst = y[k * RCH:(k + 1) * RCH].rearrange(
                "r (q j) c -> (r q) (j c)", q=PPR)
            st = nc.sync.dma_start(out=dst, in_=xk[:])
            stores.append(st)

        # ---- cross-partition reduce to per-row s, e, 1/(e-s) ----
        mt = psum.tile([2 * CHUNKS, P], F32)
        nc.tensor.transpose(out=mt[:], in_=m[:], identity=ident[:])
        mins = small.tile([2 * CHUNKS, RCH], F32)
        nc.vector.tensor_reduce(
            out=mins[:], in_=mt[:].rearrange("p (i w) -> p i w", w=PPR),
            axis=AX.X, op=OP.min)
        # rows 0-3: first_nan + 2048*i ; rows 4-7: -(last_nan + 2048*i)
        n2 = small.tile([CHUNKS, RCH], F32)
        nc.sync.dma_start(out=n2[:], in_=mins[CHUNKS:2 * CHUNKS, :])

        fn4 = mins[0:CHUNKS, :]
        pk = small.tile([CHUNKS, RCH * 4], F32)
        nc.vector.memset(pk[:], 0.0)
        pkv = pk[:].rearrange("c (i k) -> c i k", k=4)
        c2v = c2048[:].rearrange("c (i o) -> c i o", o=1)
        # s = first_nan - 2048*i - 1
        nc.vector.scalar_tensor_tensor(
            out=pkv[:, :, 0:1],
            in0=fn4.rearrange("c (i o) -> c i o", o=1), scalar=-1.0,
            in1=c2v, op0=OP.add, op1=OP.subtract)
        # e = last_nan + 1 = -(n2 + 2048*i) + 1
        t2 = small.tile([CHUNKS, RCH], F32)
        nc.vector.tensor_tensor(
            out=t2[:].rearrange("c (i o) -> c i o", o=1),
            in0=n2[:].rearrange("c (i o) -> c i o", o=1),
            in1=c2v, op=OP.add)
        nc.vector.tensor_scalar(
            out=pkv[:, :, 1:2],
            in0=t2[:].rearrange("c (i o) -> c i o", o=1),
            scalar1=-1.0, scalar2=1.0, op0=OP.mult, op1=OP.add)
        # 1 / (e - s)
        es = small.tile([CHUNKS, RCH], F32)
        nc.vector.tensor_tensor(
            out=es[:].rearrange("c (i o) -> c i o", o=1),
            in0=pkv[:, :, 1:2], in1=pkv[:, :, 0:1], op=OP.subtract)
        nc.vector.reciprocal(
            out=pkv[:, :, 2:3],
            in_=es[:].rearrange("c (i o) -> c i o", o=1))

        scr = dram.tile([R, 4], F32)
        wsc = nc.sync.dma_start(
            out=scr[:].rearrange("(c i) k -> c (i k)", c=CHUNKS), in_=pk[:])

        # ---- replicate per-row scalars to [128, .] via indirect gather ----
        g = small.tile([P, 4], F32)
        gi = nc.gpsimd.indirect_dma_start(
            out=g[:], out_offset=None, in_=scr[:],
            in_offset=bass.IndirectOffsetOnAxis(ap=idx4[:, 0:1], axis=0))
        add_dep_helper(_ins(gi), _ins(wsc), reason="gather scalars after scratch write")

        fxs = small.tile([P, 1], F32)
        nc.vector.tensor_tensor(out=fxs[:], in0=g[:, 0:1], in1=d128f[:], op=OP.add)
        ixs = small.tile([P, 1], I32)
        nc.vector.tensor_copy(out=ixs[:], in_=fxs[:])
        fxe = small.tile([P, 1], F32)
        nc.vector.tensor_tensor(out=fxe[:], in0=g[:, 1:2], in1=d128f[:], op=OP.add)
        ixe = small.tile([P, 1], I32)
        nc.vector.tensor_copy(out=ixe[:], in_=fxe[:])
        fww = small.tile([P, 1], F32)
        nc.vector.tensor_tensor(out=fww[:], in0=fxs[:], in1=w128f[:], op=OP.add)
        ixw = small.tile([P, 1], I32)
        nc.vector.tensor_copy(out=ixw[:], in_=fww[:])

        xs = small.tile([P, C], F32)
        nc.gpsimd.indirect_dma_start(
            out=xs[:], out_offset=None, in_=xv,
            in_offset=bass.IndirectOffsetOnAxis(ap=ixs[:, 0:1], axis=0))
        xe = small.tile([P, C], F32)
        nc.gpsimd.indirect_dma_start(
            out=xe[:], out_offset=None, in_=xv,
            in_offset=bass.IndirectOffsetOnAxis(ap=ixe[:, 0:1], axis=0))

        df = small.tile([P, C], F32)
        nc.vector.tensor_tensor(out=df[:], in0=xe[:], in1=xs[:], op=OP.subtract)
        slope = small.tile([P, C], F32)
        nc.vector.tensor_scalar(out=slope[:], in0=df[:], scalar1=g[:, 2:3],
                                scalar2=None, op0=OP.mult)
        base = small.tile([P, C], F32)
        # base = xs + (1 + 128*(p%4)) * slope
        nc.vector.scalar_tensor_tensor(
            out=base[:], in0=slope[:], scalar=w128f[:, 0:1], in1=xs[:],
            op0=OP.mult, op1=OP.add)

        # ---- window gather, interp, select, scatter ----
        xw = win.tile([P, WF * C], F32)
        nc.gpsimd.indirect_dma_start(
            out=xw[:], out_offset=None, in_=xv,
            in_offset=bass.IndirectOffsetOnAxis(ap=ixw[:, 0:1], axis=0))

        prod = win.tile([P, WF * C], F32)
        prod3 = prod[:].rearrange("p (f c) -> p f c", c=C)
        nc.vector.tensor_tensor(
            out=prod3, in0=_bcast_inner(fgrid[:], C),
            in1=_bcast_mid(slope[:], WF), op=OP.mult)
        nc.vector.tensor_tensor(
            out=prod3, in0=prod3, in1=_bcast_mid(base[:], WF), op=OP.add)
        vm = win.tile([P, WF * C], mybir.dt.uint8)
        nc.vector.tensor_tensor(out=vm[:], in0=xw[:], in1=xw[:], op=OP.is_equal)
        nc.vector.copy_predicated(out=prod[:], mask=vm[:], data=xw[:])

        sc = nc.gpsimd.indirect_dma_start(
            out=yv, out_offset=bass.IndirectOffsetOnAxis(ap=ixw[:, 0:1], axis=0),
            in_=prod[:], in_offset=None)
        for st in stores:
            add_dep_helper(_ins(sc), _ins(st), reason="scatter windows after bulk store")


_NC = None


def _get_nc():
    global _NC
    if _NC is None:
        nc = bacc.Bacc("TRN2", target_bir_lowering=False, debug=False,
                       num_devices=NCORES)
        x = nc.dram_tensor("x", [R, T, C], F32, kind="ExternalInput")
        y = nc.dram_tensor("y", [R, T, C], F32, kind="ExternalOutput")
        with tile.TileContext(nc) as tc:
            build_kernel(tc, x.ap(), y.ap())
        nc.compile()
        _NC = nc
    return _NC


def kernel(x):
    from concourse.bass_utils import run_bass_kernel_spmd

    x = np.ascontiguousarray(x, dtype=np.float32)
    assert x.shape == (B, T, A, D), x.shape
    xr = x.reshape(NCORES, R, T, C)
    nc = _get_nc()
    in_maps = [{"x": xr[i]} for i in range(NCORES)]
    res = run_bass_kernel_spmd(nc, in_maps, core_ids=list(range(NCORES)))
    out = np.stack([res.results[i]["y"] for i in range(NCORES)])
    return out.reshape(B, T, A, D)



# revision 33
# speedup vs baseline: 1.4360x; 1.4360x over previous
"""Trainium2 Bass kernel for nn_InterpolationModel (NaN-gap linear interpolation).

Problem: x [256, 2048, 22, 2] f32, one contiguous NaN gap along T per batch row
(s in [1, 1023], gap in [1, 511], so the gap and its +31-frame fix-up margin
always lie inside frames [2, 1597] of a 2048-frame row).
Output: x with the gap filled by linear interpolation between the last valid
frame before the gap (s) and the first valid frame after it (e).

Strategy (pure data parallel over batch, 32 rows per core, 4 chunks of 8 rows):
  - Per chunk: bulk copy x -> y_k through a [128, 5632] SBUF tile
    (16 partitions per row x 128 frames). Each chunk writes its OWN
    ExternalOutput tensor y0..y3 (concatenated on host) because Tile
    serializes all writers of a single DRAM tensor at tensor granularity -
    one shared y turns the four chunk pipelines into one serial chain.
  - Per-partition NaN scan on channel-0 samples -> TensorE transpose ->
    grouped reduce -> per-row s, e on 8 partitions. xs comes from a tiny
    [8, 44] indirect gather at frame s; frames e..e+31 are gathered as the
    [8, 1408] fix-up tile whose first 44 elements are xe.
  - Per-row payload [8, 90] (xs | slope | s | g) is broadcast to all
    128 partitions with one matmul against a constant 0/1 matrix (no DRAM
    scratch round trip).
  - The gap is written as ceil(gap/32) 32-frame blocks: partition (row, k)
    scatters interp values for block j = k mod g (conditional-subtract
    cascade in exact small-int f32), so duplicate blocks write identical
    bytes to identical addresses (harmless, order-free) and no block ever
    writes outside [s+1, s+32g]. The last block overruns the gap by <= 31
    frames; the fix-up scatter then rewrites frames [e, e+31] with original
    x values (which beyond the overrun are byte-identical to what the bulk
    copy already stored). No NaN-mask / copy_predicated on big tiles, no
    512-frame window gather: dense math is 2 vector ops on [128, 1408] per
    chunk.
  - All iotas/identity/broadcast matrices come in via a second ExternalInput
    "cst" computed on host - no gpsimd iota preamble.

Scheduling: the Tile scheduler's static per-engine orders convoy badly here,
so the kernel pins them with sync=False (order-only) add_dep_helper edges:
all loads issue before any store's load-complete wait on the Sync queue;
GpSimd runs all gathers, then all block scatters, then all fix-ups; the DVE
queue interleaves A1 (detect/scalars/offsets, emitted per chunk ASAP) with
A2 (slope/broadcast/interp) as A1_0 A1_1 A2_0 A1_2 A2_1 A1_3 A2_2 A2_3 so a
later chunk's detection never parks behind an earlier chunk's gather-stalled
interp. Real WAW on y_k (store -> scatter -> fix-up) uses sync=True edges.
Measured on TRN2: 121 us (staged baseline) -> 82.5 us.
"""

import numpy as np

import concourse.bacc as bacc
import concourse.bass as bass
import concourse.mybir as mybir
import concourse.tile as tile
from bass_rust import add_dep_helper

F32 = mybir.dt.float32
I32 = mybir.dt.int32
AX = mybir.AxisListType
OP = mybir.AluOpType

# Full problem: B=256, T=2048, A=22, D=2 over 8 cores.
B, T, A, D = 256, 2048, 22, 2
C = A * D             # 44 contiguous f32 per frame
NCORES = 8
R = B // NCORES       # 32 rows per core
CHUNKS = 4
RCH = R // CHUNKS     # 8 rows per chunk
P = 128
FPP = T // 16         # 128 frames per partition in a chunk tile
WF = 32               # frames per scatter block
BIG = 65536.0

# cst column layout
C_KK = 0              # p % 16
C_RB0 = 1             # 2048 * (p // 16)
C_RB8 = 2             # 2048 * p (rows 0..7 used)
C_TCH = 4             # [4, 132): tchunk[p, j] = (p%16)*128 + j
C_FG = 132            # [132, 164): fgrid[p, j] = j
C_ID = 164            # [164, 292): identity
C_BC = 292            # [292, 420): bc16[r, p] = (p//16 == r), rows 0..7 used
C_THR = 420           # [420, 436): 32*i + 1.5 (g = #{i: d > thr_i})
C_MUL = 436           # [436, 440): 8, 4, 2, 1 (mod cascade multipliers)
NCOL = 440


def _bcast2(ap, count):
    """[P, 1] AP -> [P, count] with a stride-0 free axis."""
    return bass.AP(ap.tensor, ap.offset, [list(ap.ap[0]), [0, count]])


def _bcast_mid(ap, count):
    """[P, n] AP -> [P, count, n] with a stride-0 middle axis."""
    return bass.AP(ap.tensor, ap.offset, [list(ap.ap[0]), [0, count], list(ap.ap[1])])


def _bcast_inner(ap, count):
    """[P, n] AP -> [P, n, count] with a stride-0 inner axis."""
    return bass.AP(ap.tensor, ap.offset, [list(ap.ap[0]), list(ap.ap[1]), [0, count]])


def _ins(bi):
    return bi.ins if hasattr(bi, "ins") else bi


def build_kernel(tc, x, ys, cst):
    nc = tc.nc
    xv = x.rearrange("b t c -> (b t) c")   # [R*T, C] frame rows, offset 0

    from contextlib import ExitStack

    with ExitStack() as ctx:
        const = ctx.enter_context(tc.tile_pool(name="const", bufs=1))
        adata = ctx.enter_context(tc.tile_pool(name="adata", bufs=CHUNKS))
        work = ctx.enter_context(tc.tile_pool(name="work", bufs=CHUNKS))
        small = ctx.enter_context(tc.tile_pool(name="small", bufs=CHUNKS))
        psum = ctx.enter_context(tc.tile_pool(name="psum", bufs=2, space="PSUM"))

        cs = const.tile([P, NCOL], F32)
        nc.sync.dma_start(out=cs[:], in_=cst)
        kk = cs[:, C_KK:C_KK + 1]
        rowb0 = cs[:, C_RB0:C_RB0 + 1]
        rowb8 = cs[0:RCH, C_RB8:C_RB8 + 1]
        tchunk = cs[:, C_TCH:C_TCH + FPP]
        fgrid = cs[:, C_FG:C_FG + WF]
        ident = cs[:, C_ID:C_ID + P]
        bc16 = cs[0:RCH, C_BC:C_BC + P]
        thr8 = cs[0:RCH, C_THR:C_THR + 16]
        cmul = cs[:, C_MUL:C_MUL + 4]

        deferred = []  # (yk_view, oi, interp, osloc_i, xfix, store_ins) per chunk
        lds, sts, gths = [], [], []

        def phase_a1(k):
            yk = ys[k]                     # [RCH, T, C] chunk output tensor
            # ---- bulk copy: 8 rows as [128, 5632] ----
            xk = adata.tile([P, FPP * C], F32, tag="xk")
            ld = nc.sync.dma_start(
                out=xk[:],
                in_=x[k * RCH:(k + 1) * RCH].rearrange(
                    "r (q j) c -> (r q) (j c)", q=16))
            sta = nc.sync.dma_start(
                out=yk.rearrange("r (q j) c -> (r q) (j c)", q=16), in_=xk[:])
            lds.append(ld)
            sts.append(sta)

            # ---- per-partition NaN scan on channel-0 samples ----
            samp = xk[:].rearrange("p (j c) -> p j c", c=C)[:, :, 0:1]
            v = work.tile([P, FPP], F32, tag="v")
            a1_first = nc.vector.tensor_tensor(
                out=v[:].rearrange("p (j o) -> p j o", o=1),
                in0=samp, in1=samp, op=OP.is_equal)
            bb = work.tile([P, FPP], F32, tag="bb")
            # valid*BIG + t : min = first NaN frame (per partition)
            nc.vector.scalar_tensor_tensor(
                out=bb[:], in0=v[:], scalar=BIG, in1=tchunk,
                op0=OP.mult, op1=OP.add)
            m = small.tile([P, 2], F32, tag="m")
            nc.vector.tensor_reduce(out=m[:, 0:1], in_=bb[:], axis=AX.X, op=OP.min)
            dd = work.tile([P, FPP], F32, tag="dd")
            # (valid-1)*t = -(t on NaN frames) : min = -last NaN frame
            nc.vector.scalar_tensor_tensor(
                out=dd[:], in0=v[:], scalar=-1.0, in1=tchunk,
                op0=OP.add, op1=OP.mult)
            nc.vector.tensor_reduce(out=m[:, 1:2], in_=dd[:], axis=AX.X, op=OP.min)

            # ---- cross-partition: [128, 2] -> per-row scalars on 8 partitions
            mt = psum.tile([2, P], F32, tag="mt")
            nc.tensor.transpose(out=mt[:], in_=m[:], identity=ident)
            mins2 = small.tile([2, RCH], F32, tag="mins2")
            nc.vector.tensor_reduce(
                out=mins2[:], in_=mt[:].rearrange("c (r q) -> c r q", q=16),
                axis=AX.X, op=OP.min)
            rsc = psum.tile([RCH, 2], F32, tag="rsc")
            nc.tensor.transpose(out=rsc[:], in_=mins2[:], identity=ident[0:2, 0:2])

            # payload [8, 90]: xs(44) | slope(44) | s | g
            pl = small.tile([RCH, 90], F32, tag="pl")
            s8 = pl[:, 88:89]
            nc.vector.tensor_scalar(out=s8, in0=rsc[:, 0:1], scalar1=1.0,
                                    scalar2=None, op0=OP.subtract)
            e8 = small.tile([RCH, 2], F32, tag="e8")
            # e = -(-last) + 1
            nc.vector.tensor_scalar(out=e8[:, 0:1], in0=rsc[:, 1:2], scalar1=-1.0,
                                    scalar2=1.0, op0=OP.mult, op1=OP.add)

            # gather offsets into x (global: + k*RCH*T); scatter offsets into
            # yk are chunk-local (rowb8 only). Computed first so the gathers
            # launch as early as possible.
            osf = small.tile([RCH, 2], F32, tag="osf")
            nc.vector.scalar_tensor_tensor(
                out=osf[:, 0:1], in0=s8, scalar=float(k * RCH * T), in1=rowb8,
                op0=OP.add, op1=OP.add)
            nc.vector.scalar_tensor_tensor(
                out=osf[:, 1:2], in0=e8[:, 0:1], scalar=float(k * RCH * T),
                in1=rowb8, op0=OP.add, op1=OP.add)
            osi = small.tile([RCH, 2], I32, tag="osi")
            nc.vector.tensor_copy(out=osi[:], in_=osf[:])
            oslf = small.tile([RCH, 1], F32, tag="oslf")
            nc.vector.tensor_tensor(out=oslf[:], in0=e8[:, 0:1], in1=rowb8,
                                    op=OP.add)
            osloc = small.tile([RCH, 1], I32, tag="osloc")
            nc.vector.tensor_copy(out=osloc[:], in_=oslf[:])

            d8 = small.tile([RCH, 2], F32, tag="d8")
            nc.vector.tensor_tensor(out=d8[:, 0:1], in0=e8[:, 0:1], in1=s8,
                                    op=OP.subtract)
            inv8 = small.tile([RCH, 1], F32, tag="inv8")
            nc.vector.reciprocal(out=inv8[:], in_=d8[:, 0:1])
            # g = ceil((d-1)/32) = #{i in 0..15 : d > 32i + 1.5}  (exact ints)
            cgt = small.tile([RCH, 16], F32, tag="cgt")
            nc.vector.tensor_tensor(
                out=cgt[:], in0=_bcast2(d8[:, 0:1], 16), in1=thr8, op=OP.is_gt)
            a1_last = nc.vector.tensor_reduce(out=pl[:, 89:90], in_=cgt[:],
                                              axis=AX.X, op=OP.add)

            # xs gather [8, 44] straight into the payload; xe gathered
            # separately so slope never waits for the bulky fix-up gather
            ga = nc.gpsimd.indirect_dma_start(
                out=pl[:, 0:44], out_offset=None, in_=xv,
                in_offset=bass.IndirectOffsetOnAxis(ap=osi[:, 0:1], axis=0))
            xe8 = small.tile([RCH, C], F32, tag="xe8")
            gb = nc.gpsimd.indirect_dma_start(
                out=xe8[:], out_offset=None, in_=xv,
                in_offset=bass.IndirectOffsetOnAxis(ap=osi[:, 1:2], axis=0))
            gths.extend([ga, gb])
            return dict(yk=yk, pl=pl, s8=s8, e8=e8, inv8=inv8, osi=osi,
                        osloc=osloc, xe8=xe8, sta=sta, k=k,
                        first=a1_first, last=a1_last)

        def phase_a2(st):
            k, yk, pl, inv8 = st["k"], st["yk"], st["pl"], st["inv8"]
            osi, osloc, sta = st["osi"], st["osloc"], st["sta"]
            # slope = (xe - xs) / (e - s)
            df8 = small.tile([RCH, C], F32, tag="df8")
            a2_first = nc.vector.tensor_tensor(out=df8[:], in0=st["xe8"][:],
                                               in1=pl[:, 0:44], op=OP.subtract)
            nc.vector.tensor_scalar(out=pl[:, 44:88], in0=df8[:],
                                    scalar1=inv8[:, 0:1], scalar2=None, op0=OP.mult)

            # ---- broadcast payload to 128 partitions via matmul ----
            bcp = psum.tile([P, 90], F32, tag="bcp")
            nc.tensor.matmul(bcp[:], bc16, pl[:], start=True, stop=True)
            bc = small.tile([P, 90], F32, tag="bc")
            nc.vector.tensor_copy(out=bc[:], in_=bcp[:])
            xs128 = bc[:, 0:44]
            sl128 = bc[:, 44:88]
            s128 = bc[:, 88:89]
            g128 = bc[:, 89:90]

            # j = kk mod g via conditional-subtract cascade (exact int f32);
            # gmul[:, i] = g * (8 >> i)
            gmul = small.tile([P, 4], F32, tag="gmul")
            nc.vector.tensor_tensor(out=gmul[:], in0=_bcast2(g128, 4),
                                    in1=cmul, op=OP.mult)
            jprev = kk
            for i in range(4):
                gci = gmul[:, i:i + 1]
                dsub = small.tile([P, 1], F32, tag=f"dsub{i}")
                # (jprev >= g*2^i) * g*2^i
                nc.vector.scalar_tensor_tensor(out=dsub[:], in0=jprev,
                                               scalar=gci, in1=gci,
                                               op0=OP.is_ge, op1=OP.mult)
                jnew = small.tile([P, 1], F32, tag=f"jnew{i}")
                nc.vector.tensor_tensor(out=jnew[:], in0=jprev, in1=dsub[:],
                                        op=OP.subtract)
                jprev = jnew[:]
            wf = small.tile([P, 1], F32, tag="wf")     # W = 32*j + 1
            nc.vector.tensor_scalar(out=wf[:], in0=jprev, scalar1=32.0,
                                    scalar2=1.0, op0=OP.mult, op1=OP.add)

            # chunk-local scatter offset = rowb0 + s + W
            of2 = small.tile([P, 1], F32, tag="of2")
            nc.vector.scalar_tensor_tensor(out=of2[:], in0=wf[:],
                                           scalar=s128, in1=rowb0,
                                           op0=OP.add, op1=OP.add)
            oi = small.tile([P, 1], I32, tag="oi")
            nc.vector.tensor_copy(out=oi[:], in_=of2[:])

            # base = xs + W*slope ; interp[p, j, c] = j*slope + base
            base = small.tile([P, C], F32, tag="base")
            nc.vector.scalar_tensor_tensor(out=base[:], in0=sl128,
                                           scalar=wf[:, 0:1], in1=xs128,
                                           op0=OP.mult, op1=OP.add)
            interp = work.tile([P, WF * C], F32, tag="interp")
            prod3 = interp[:].rearrange("p (f c) -> p f c", c=C)
            nc.vector.tensor_tensor(
                out=prod3, in0=_bcast_inner(fgrid, C),
                in1=_bcast_mid(sl128, WF), op=OP.mult)
            a2_last = nc.vector.tensor_tensor(
                out=prod3, in0=prod3, in1=_bcast_mid(base[:], WF), op=OP.add)
            st["first"], st["last"] = a2_first, a2_last

            deferred.append((yk.rearrange("b t c -> (b t) c"), oi, interp,
                             osloc, osi, sta))

        # interleave: every chunk's A1 (detect/scalars/gathers) is emitted
        # before earlier chunks' A2 (slope/broadcast/interp) so the in-order
        # DVE queue never parks a later chunk's detection behind an earlier
        # chunk's gather-stalled interp chain.
        dve_chain = []

        def link(node):
            if dve_chain:
                add_dep_helper(_ins(node["first"]), _ins(dve_chain[-1]["last"]),
                               sync=False, reason="DVE queue order")
            dve_chain.append(dict(node))

        sts_a = [phase_a1(0), phase_a1(1)]
        link(sts_a[0]); link(sts_a[1])
        phase_a2(sts_a[0]); link(sts_a[0])
        sts_a.append(phase_a1(2)); link(sts_a[2])
        phase_a2(sts_a[1]); link(sts_a[1])
        sts_a.append(phase_a1(3)); link(sts_a[3])
        phase_a2(sts_a[2]); link(sts_a[2])
        phase_a2(sts_a[3]); link(sts_a[3])

        # ---- engine-queue ordering (sync=False = order-only, no semaphore):
        # Sync: all loads issue before any store's load-complete wait.
        for a, b in zip(lds[1:], lds[:-1]):
            add_dep_helper(_ins(a), _ins(b), sync=False, reason="load order")
        add_dep_helper(_ins(sts[0]), _ins(lds[-1]), sync=False,
                       reason="stores queue after all loads")
        for a, b in zip(sts[1:], sts[:-1]):
            add_dep_helper(_ins(a), _ins(b), sync=False, reason="store order")
        # GpSimd: all gathers, then all block scatters, then all fix-ups —
        # a scatter's store-wait must not block a later chunk's gathers.
        for a, b in zip(gths[1:], gths[:-1]):
            add_dep_helper(_ins(a), _ins(b), sync=False, reason="gather order")

        # ---- phase B: fix-up gathers (frames e..e+31, only needed by the
        # final fix-up scatters), then all block scatters, then all fix-ups.
        prev = gths[-1]
        xfixes = []
        for ykv, oi, interp, osloc, osi, sta in deferred:
            xfix = work.tile([RCH, WF * C], F32, tag="xfix")
            gf = nc.gpsimd.indirect_dma_start(
                out=xfix[:], out_offset=None, in_=xv,
                in_offset=bass.IndirectOffsetOnAxis(ap=osi[:, 1:2], axis=0))
            add_dep_helper(_ins(gf), _ins(prev), sync=False, reason="gather order")
            prev = gf
            xfixes.append(xfix)
        scs = []
        for ykv, oi, interp, osloc, osi, sta in deferred:
            sc = nc.gpsimd.indirect_dma_start(
                out=ykv, out_offset=bass.IndirectOffsetOnAxis(ap=oi[:, 0:1], axis=0),
                in_=interp[:], in_offset=None)
            add_dep_helper(_ins(sc), _ins(sta), reason="scatter after bulk store")
            add_dep_helper(_ins(sc), _ins(prev), sync=False, reason="scatter order")
            prev = sc
            scs.append(sc)
        for (ykv, oi, interp, osloc, osi, sta), sc, xfix in zip(deferred, scs,
                                                                xfixes):
            fx = nc.gpsimd.indirect_dma_start(
                out=ykv, out_offset=bass.IndirectOffsetOnAxis(ap=osloc[:, 0:1],
                                                              axis=0),
                in_=xfix[:], in_offset=None)
            add_dep_helper(_ins(fx), _ins(sc), reason="fix-up after block scatter")
            add_dep_helper(_ins(fx), _ins(prev), sync=False, reason="fix-up order")
            prev = fx


def _make_cst() -> np.ndarray:
    cst = np.zeros((P, NCOL), dtype=np.float32)
    p = np.arange(P)
    cst[:, C_KK] = p % 16
    cst[:, C_RB0] = T * (p // 16)
    cst[:, C_RB8] = T * p
    cst[:, C_TCH:C_TCH + FPP] = (p % 16)[:, None] * FPP + np.arange(FPP)[None, :]
    cst[:, C_FG:C_FG + WF] = np.arange(WF)[None, :]
    cst[:, C_ID:C_ID + P] = np.eye(P, dtype=np.float32)
    cst[0:RCH, C_BC:C_BC + P] = (p[None, :] // 16 == np.arange(RCH)[:, None])
    cst[:, C_THR:C_THR + 16] = 32.0 * np.arange(16)[None, :] + 1.5
    cst[:, C_MUL:C_MUL + 4] = np.array([8.0, 4.0, 2.0, 1.0])[None, :]
    return cst


_NC = None


def _get_nc():
    global _NC
    if _NC is None:
        nc = bacc.Bacc("TRN2", target_bir_lowering=False, debug=False,
                       num_devices=NCORES)
        x = nc.dram_tensor("x", [R, T, C], F32, kind="ExternalInput")
        cst = nc.dram_tensor("cst", [P, NCOL], F32, kind="ExternalInput")
        ys = [nc.dram_tensor(f"y{k}", [RCH, T, C], F32, kind="ExternalOutput")
              for k in range(CHUNKS)]
        with tile.TileContext(nc) as tc:
            build_kernel(tc, x.ap(), [yk.ap() for yk in ys], cst.ap())
        nc.compile()
        _NC = nc
    return _NC


def _in_maps(x: np.ndarray) -> list:
    xr = x.reshape(NCORES, R, T, C)
    cst = _make_cst()
    return [{"x": xr[i], "cst": cst} for i in range(NCORES)]


def kernel(x):
    from concourse.bass_utils import run_bass_kernel_spmd

    x = np.ascontiguousarray(x, dtype=np.float32)
    assert x.shape == (B, T, A, D), x.shape
    nc = _get_nc()
    res = run_bass_kernel_spmd(nc, _in_maps(x), core_ids=list(range(NCORES)))
    out = np.stack([
        np.concatenate([res.results[i][f"y{k}"] for k in range(CHUNKS)], axis=0)
        for i in range(NCORES)])
    return out.reshape(B, T, A, D)
